# revision 1
# baseline (speedup 1.0000x reference)
"""Trainium2 Bass kernel for nn_DMLNegHead (retrieval_knn head).

Strategy: data-parallel over batch B=16 across 8 NeuronCores (2 images per
core), prototype/offset params replicated. No collectives needed — every
output has a leading batch axis.

Per-core pipeline, processed in spatial chunks of 512 positions:
  conv(1x1) -> PSUM; square+ones-matmul -> ||emb||^2; rnorm = exp(-0.5 ln ssq);
  partition-broadcast rnorm (GPSIMD); emb_n = emb * rnorm (DVE);
  dot products against 1095 prototype rows (PE);
  all transcendentals via the single ACT table set {ln, exp, square}:
    dist = exp(0.5 ln(c - 2 dot)),  probs_* = exp(affine(dot)),
  min/mul/add chain on DVE/GPSIMD, class-sum via ones-matmul, cls = probs/sum.
"""
import sys
sys.path.insert(0, "/opt/trn_rl_repo")

import numpy as np
import concourse.bass as bass
import concourse.tile as tile
from concourse import bacc, mybir
from concourse.bass_utils import run_bass_kernel_spmd

F32 = mybir.dt.float32
F32R = mybir.dt.float32r
AF = mybir.ActivationFunctionType
ALU = mybir.AluOpType

# problem constants (hardcoded per contract)
B, CIN, H, W = 16, 256, 64, 64
E, R, NEG = 256, 365, 2
NCORES = 8
BL = B // NCORES          # batches per core
N = H * W                 # 4096 spatial positions
NCH = 512                 # chunk of positions per inner step
NJ = N // NCH
SIGMA, BETA = 0.5, 0.3
INV2S2 = 1.0 / (2.0 * SIGMA ** 2)   # 2.0

# matmul dtype for the heavy matmuls: 'f32' (exact, 4 cyc/row) or
# 'f32r' (tf32-like, 1 cyc/row).
DT_MM_DEFAULT = "f32"

# slabs of prototype rows: (row0, nrows); 365 = 128 + 128 + 109
SLABS = [(0, 128), (128, 128), (256, R - 256)]

LAST_EXEC_TIME_NS = None


def _build(dt_mm: str):
    DTM = F32R if dt_mm == "f32r" else F32
    nc = bacc.Bacc("TRN2", target_bir_lowering=False)

    x_d = nc.dram_tensor("x", [BL, CIN, N], F32, kind="ExternalInput")
    convT_d = nc.dram_tensor("convT", [CIN, E], F32, kind="ExternalInput")
    convb_d = nc.dram_tensor("convb", [1, E], F32, kind="ExternalInput")
    repsT_d = nc.dram_tensor("repsT", [E, 3 * R], F32, kind="ExternalInput")
    bias_d = nc.dram_tensor("biases", [128, 12], F32, kind="ExternalInput")

    o_cls = nc.dram_tensor("o_cls", [BL, R, N], F32, kind="ExternalOutput")
    o_clsneg = nc.dram_tensor("o_clsneg", [BL, R, N], F32, kind="ExternalOutput")
    o_dist = nc.dram_tensor("o_dist", [BL, R, N], F32, kind="ExternalOutput")
    o_distn = nc.dram_tensor("o_distn", [BL, R, NEG, N], F32, kind="ExternalOutput")
    o_pori = nc.dram_tensor("o_pori", [BL, R, N], F32, kind="ExternalOutput")

    with tile.TileContext(nc) as tc:
        with (
            tc.tile_pool(name="const", bufs=1) as const,
            tc.tile_pool(name="io", bufs=3) as io,
            tc.tile_pool(name="mid", bufs=3) as mid,
            tc.tile_pool(name="lnp", bufs=4) as lnp,
            tc.tile_pool(name="dout", bufs=4) as dout,
            tc.tile_pool(name="dnout", bufs=8) as dnout,
            tc.tile_pool(name="ch", bufs=4) as chp,
            tc.tile_pool(name="ps_emb", bufs=3, space="PSUM") as ps_emb,
            tc.tile_pool(name="ps_small", bufs=2, space="PSUM") as ps_small,
            tc.tile_pool(name="ps_dot", bufs=3, space="PSUM") as ps_dot,
        ):
            # ---- resident constants ----
            convT_f = const.tile([128, 2, E], F32)      # [k, 2 ktiles, E]
            nc.sync.dma_start(out=convT_f[:],
                              in_=convT_d[:].rearrange("(a k) e -> k a e", k=128))
            convb_f = const.tile([1, E], F32)
            nc.sync.dma_start(out=convb_f[:], in_=convb_d[:])
            repsT_f = const.tile([128, 2, 3 * R], F32)
            nc.sync.dma_start(out=repsT_f[:],
                              in_=repsT_d[:].rearrange("(a k) r -> k a r", k=128))
            biases = const.tile([128, 12], F32)
            nc.sync.dma_start(out=biases[:], in_=bias_d[:])

            ones_f = const.tile([128, 1], F32)
            nc.vector.memset(ones_f[:], 1.0)
            onesr_f = const.tile([1, NCH], F32)
            nc.vector.memset(onesr_f[:], 1.0)

            if DTM is F32R:
                convT_s = const.tile([128, 2, E], F32R)
                nc.vector.tensor_copy(convT_s[:], convT_f[:])
                convb_s = const.tile([1, E], F32R)
                nc.vector.tensor_copy(convb_s[:], convb_f[:])
                repsT_s = const.tile([128, 2, 3 * R], F32R)
                nc.vector.tensor_copy(repsT_s[:], repsT_f[:])
                ones_c = const.tile([128, 1], F32R)
                nc.vector.tensor_copy(ones_c[:], ones_f[:])
                ones_r = const.tile([1, NCH], F32R)
                nc.vector.tensor_copy(ones_r[:], onesr_f[:])
            else:
                convT_s, convb_s, repsT_s = convT_f, convb_f, repsT_f
                ones_c, ones_r = ones_f, onesr_f

            # bias column layout in `biases`:
            #  col g*3+s      : ln bias c = 1 + rn2  for group g, slab s
            #  col 9+s        : exp bias -INV2S2*c   for ori slab s
            for b in range(BL):
                for j in range(NJ):
                    ns = slice(j * NCH, (j + 1) * NCH)

                    # -- load x chunk (2 k-tiles) --
                    xs = []
                    for k in range(2):
                        xf = io.tile([128, NCH], F32, tag=f"x{k}")
                        nc.sync.dma_start(
                            out=xf[:], in_=x_d[b, k * 128:(k + 1) * 128, ns])
                        if DTM is F32R:
                            xr = io.tile([128, NCH], F32R, tag=f"xr{k}")
                            nc.gpsimd.tensor_copy(out=xr[:], in_=xf[:])
                            xs.append(xr)
                        else:
                            xs.append(xf)

                    # -- conv: emb[e, n] in PSUM (2 e-tiles) --
                    emb_ps = []
                    for m in range(2):
                        ep = ps_emb.tile([128, NCH], F32, tag="emb")
                        mcols = slice(m * 128, (m + 1) * 128)
                        nc.tensor.matmul(ep[:], convT_s[:, 0, mcols], xs[0][:],
                                         start=True, stop=False)
                        nc.tensor.matmul(ep[:], convT_s[:, 1, mcols], xs[1][:],
                                         start=False, stop=False)
                        nc.tensor.matmul(ep[:], convb_s[:, mcols], ones_r[:],
                                         start=False, stop=True)
                        emb_ps.append(ep)

                    # -- ssq via square + ones-matmul --
                    sqs = []
                    for m in range(2):
                        sq = mid.tile([128, NCH], DTM, tag=f"sq{m}")
                        if DTM is F32R:
                            nc.vector.tensor_mul(sq[:], emb_ps[m][:], emb_ps[m][:])
                        else:
                            nc.scalar.activation(sq[:], emb_ps[m][:], AF.Square)
                        sqs.append(sq)
                    ssq = ps_small.tile([1, NCH], F32, tag="small")
                    nc.tensor.matmul(ssq[:], ones_c[:], sqs[0][:],
                                     start=True, stop=False)
                    nc.tensor.matmul(ssq[:], ones_c[:], sqs[1][:],
                                     start=False, stop=True)

                    # -- rnorm = ssq^-0.5 = exp(-0.5 ln(ssq)) --
                    lnssq = mid.tile([1, NCH], F32, tag="lnssq")
                    nc.scalar.activation(lnssq[:], ssq[:], AF.Ln)
                    rnorm = mid.tile([1, NCH], F32, tag="rnorm")
                    nc.scalar.activation(rnorm[:], lnssq[:], AF.Exp, scale=-0.5)
                    bcast = mid.tile([128, NCH], F32, tag="bcast")
                    nc.gpsimd.partition_broadcast(bcast[:], rnorm[:])

                    # -- emb_n = emb * rnorm --
                    embn = []
                    for m in range(2):
                        en = mid.tile([128, NCH], DTM, tag=f"embn{m}")
                        nc.vector.tensor_mul(en[:], emb_ps[m][:], bcast[:])
                        embn.append(en)

                    # -- dots + ACT chains --
                    disto_t = [None] * 3
                    distn_t = [[None] * 3, [None] * 3]
                    for g in range(3):          # 0=ori, 1=neg m0, 2=neg m1
                        for s, (r0, p) in enumerate(SLABS):
                            col0 = g * R + r0
                            dp = ps_dot.tile([128, NCH], F32, tag="dot")
                            nc.tensor.matmul(
                                dp[:p, :], repsT_s[:, 0, col0:col0 + p],
                                embn[0][:], start=True, stop=False)
                            nc.tensor.matmul(
                                dp[:p, :], repsT_s[:, 1, col0:col0 + p],
                                embn[1][:], start=False, stop=True)

                            bc = g * 3 + s
                            lnd = lnp.tile([128, NCH], F32, tag="lnd")
                            nc.scalar.activation(
                                lnd[:p, :], dp[:p, :], AF.Ln,
                                bias=biases[:p, bc:bc + 1], scale=-2.0)
                            if g == 0:
                                dist = dout.tile([128, NCH], F32, tag="disto")
                                nc.scalar.activation(dist[:p, :], lnd[:p, :],
                                                     AF.Exp, scale=0.5)
                                nc.sync.dma_start(
                                    out=o_dist[b, r0:r0 + p, ns], in_=dist[:p, :])
                                disto_t[s] = dist
                                pori = dout.tile([128, NCH], F32, tag="pori")
                                nc.scalar.activation(
                                    pori[:p, :], dp[:p, :], AF.Exp,
                                    bias=biases[:p, 9 + s:10 + s],
                                    scale=2.0 * INV2S2)
                                nc.sync.dma_start(
                                    out=o_pori[b, r0:r0 + p, ns], in_=pori[:p, :])
                            else:
                                dist = dnout.tile([128, NCH], F32, tag="distn")
                                nc.scalar.activation(dist[:p, :], lnd[:p, :],
                                                     AF.Exp, scale=0.5)
                                nc.sync.dma_start(
                                    out=o_distn[b, r0:r0 + p, g - 1, ns],
                                    in_=dist[:p, :])
                                distn_t[g - 1][s] = dist

                    # -- per-slab tail: min, cls_neg, t, probs --
                    probs_t = [None] * 3
                    psum = ps_small.tile([1, NCH], F32, tag="small")
                    for s, (r0, p) in enumerate(SLABS):
                        dnmin = chp.tile([128, NCH], F32, tag="dnmin")
                        nc.vector.tensor_tensor(
                            dnmin[:p, :], distn_t[0][s][:p, :],
                            distn_t[1][s][:p, :], op=ALU.min)
                        dn2 = chp.tile([128, NCH], F32, tag="dn2")
                        nc.gpsimd.tensor_mul(dn2[:p, :], dnmin[:p, :], dnmin[:p, :])
                        clsneg = chp.tile([128, NCH], F32, tag="clsneg")
                        nc.scalar.activation(clsneg[:p, :], dn2[:p, :], AF.Exp,
                                             scale=-INV2S2)
                        nc.sync.dma_start(
                            out=o_clsneg[b, r0:r0 + p, ns], in_=clsneg[:p, :])

                        st = chp.tile([128, NCH], F32, tag="st")
                        nc.gpsimd.tensor_scalar(
                            out=st[:p, :], in0=dnmin[:p, :],
                            scalar1=2.0, scalar2=-BETA,
                            op0=ALU.subtract, op1=ALU.mult)
                        t = chp.tile([128, NCH], F32, tag="t")
                        nc.vector.tensor_add(t[:p, :], disto_t[s][:p, :], st[:p, :])
                        t2 = chp.tile([128, NCH], F32, tag="t2")
                        nc.vector.tensor_mul(t2[:p, :], t[:p, :], t[:p, :])
                        probs = chp.tile([128, NCH], DTM, tag="probs")
                        nc.scalar.activation(probs[:p, :], t2[:p, :], AF.Exp,
                                             scale=-INV2S2)
                        if DTM is F32R:
                            probsr = chp.tile([128, NCH], F32R, tag="probsr")
                            nc.vector.tensor_copy(probsr[:p, :], probs[:p, :])
                            pr = probsr
                        else:
                            pr = probs
                        probs_t[s] = pr
                        nc.tensor.matmul(psum[:], ones_c[:p, :], pr[:p, :],
                                         start=(s == 0), stop=(s == 2))

                    rsum = mid.tile([1, NCH], F32, tag="rsum")
                    nc.vector.reciprocal(rsum[:], psum[:])
                    rbc = mid.tile([128, NCH], F32, tag="rbc")
                    nc.gpsimd.partition_broadcast(rbc[:], rsum[:])
                    for s, (r0, p) in enumerate(SLABS):
                        cls = chp.tile([128, NCH], F32, tag="cls")
                        nc.vector.tensor_mul(cls[:p, :], probs_t[s][:p, :],
                                             rbc[:p, :])
                        nc.sync.dma_start(
                            out=o_cls[b, r0:r0 + p, ns], in_=cls[:p, :])
    nc.compile()
    return nc


_NC_CACHE = {}


def _host_prep(x, conv_w, conv_b, representations, neg_w, neg_b):
    f = np.float32
    x = np.asarray(x, f)
    conv_w = np.asarray(conv_w, f)
    conv_b = np.asarray(conv_b, f)
    reps = np.asarray(representations, f)
    neg_w = np.asarray(neg_w, f)
    neg_b = np.asarray(neg_b, f)

    r0 = reps[:, 0, :]                                     # [R, E]
    off = (np.abs(r0) @ neg_w.T + neg_b).reshape(R, NEG, E).astype(f)
    rneg = ((off + np.abs(reps)) * np.sign(reps)).astype(f)
    nrm = np.sqrt((rneg * rneg).sum(2, keepdims=True, dtype=f))
    rneg = (rneg / np.maximum(nrm, 1e-12)).astype(f)

    # repsT: [E, 3R] columns = [ori | neg m0 | neg m1]
    allr = np.concatenate([r0[None], rneg[:, 0][None], rneg[:, 1][None]], 0)
    repsT = np.ascontiguousarray(
        allr.reshape(3 * R, E).T).astype(f)                # [E, 3R]
    rn2 = (allr * allr).sum(2, dtype=f)                    # [3, R]

    biases = np.zeros((128, 12), f)
    for g in range(3):
        for s, (rr, p) in enumerate(SLABS):
            c = (1.0 + rn2[g, rr:rr + p]).astype(f)
            biases[:p, g * 3 + s] = c
            if g == 0:
                biases[:p, 9 + s] = -INV2S2 * c

    convT = np.ascontiguousarray(conv_w.T).astype(f)       # [CIN, E]
    convb2 = conv_b.reshape(1, E).astype(f)

    shared = {"convT": convT, "convb": convb2, "repsT": repsT, "biases": biases}
    in_maps = []
    for i in range(NCORES):
        m = dict(shared)
        m["x"] = np.ascontiguousarray(
            x[i * BL:(i + 1) * BL].reshape(BL, CIN, N))
        in_maps.append(m)
    return in_maps


def _run(inputs, dt_mm=DT_MM_DEFAULT, trace=False):
    global LAST_EXEC_TIME_NS
    in_maps = _host_prep(**inputs)
    if dt_mm not in _NC_CACHE:
        _NC_CACHE[dt_mm] = _build(dt_mm)
    nc = _NC_CACHE[dt_mm]
    res = run_bass_kernel_spmd(nc, in_maps, list(range(NCORES)), trace=trace)
    LAST_EXEC_TIME_NS = res.exec_time_ns

    def cat(name):
        return np.concatenate([res.results[i][name] for i in range(NCORES)], 0)

    cls_score = cat("o_cls").reshape(B, R, H, W)
    cls_neg = cat("o_clsneg").reshape(B, R, H, W)
    distance = cat("o_dist").reshape(B, R, 1, H, W)
    distance_neg = cat("o_distn").reshape(B, R, NEG, H, W)
    probs_ori = cat("o_pori").reshape(B, R, H, W)
    return cls_score, cls_neg, distance, distance_neg, probs_ori


def kernel(**inputs):
    return _run(inputs, trace=False)


if __name__ == "__main__":
    print("kernel module; use test.py")
